# revision 1
# baseline (speedup 1.0000x reference)
"""Trainium2 Bass kernel for nn_DisOrFuncf_34067680591904.

Mathematical note: the reference computes
    out = inner + stop_gradient(fout - inner)
whose *value* is exactly fout (the `inner`/GOGradX machinery only shapes
gradients). fout is a 3-layer MLP (784 -> 512 -> 256 -> 1, leaky-relu 0.2,
sigmoid) applied to x[:, 0, :].  The eval path (is_train_g == 0) applies the
same MLP to every (batch, level) row of x.

Strategy: pure data parallelism — shard the MLP rows across the 8 cores
(32 rows/core for train, 128 rows/core for eval); weights are replicated.
On-chip layout keeps neurons on partitions and batch rows on the free dim,
so biases and leaky-relu fold naturally and no on-chip transposes are
needed.  All host-side packing produces contiguous DMAs.

Per-core dataflow (R = rows per core):
  xT chunks   [128, R] x 6 + [17, R] (row 16 of the tail = ones -> bias row)
  L1: 4 m-chunks x 7 k-chunks matmuls -> psum [128, R]; leaky-relu on DVE
  L2: 2 m-chunks x (4 k-chunks + bias row) -> psum [128, R]; leaky-relu
  L3: 2 k-chunks + bias row -> psum [1, R]; sigmoid on ACT
"""

import numpy as np

N_CORES = 8
BATCH, NC_LVL, D_IN, D_H1, D_H2 = 256, 4, 784, 512, 256

_compiled = {}  # rows_per_core -> (nc, names)


def _build_nc(R: int):
    import concourse.bacc as bacc
    import concourse.tile as tile
    from concourse import mybir

    f32 = mybir.dt.float32
    nc = bacc.Bacc("TRN2", target_bir_lowering=False, debug=False,
                   num_devices=N_CORES)

    xtm_d = nc.dram_tensor("xtm", [128, 6 * R], f32, kind="ExternalInput")
    xtt_d = nc.dram_tensor("xtt", [17, R], f32, kind="ExternalInput")
    w1m_d = nc.dram_tensor("w1m", [4, 128, 768], f32, kind="ExternalInput")
    w1t_d = nc.dram_tensor("w1t", [17, 512], f32, kind="ExternalInput")
    w2m_d = nc.dram_tensor("w2m", [2, 128, 512], f32, kind="ExternalInput")
    b2x_d = nc.dram_tensor("b2x", [1, 257], f32, kind="ExternalInput")
    w3x_d = nc.dram_tensor("w3x", [128, 2], f32, kind="ExternalInput")
    out_d = nc.dram_tensor("out", [1, R], f32, kind="ExternalOutput")

    with tile.TileContext(nc) as tc:
        with (
            tc.tile_pool(name="const", bufs=1) as cpool,
            tc.tile_pool(name="work", bufs=2) as wpool,
            tc.tile_pool(name="psum", bufs=2, space="PSUM") as ppool,
        ):
            xtm = cpool.tile([128, 6 * R], f32, tag="xtm")
            nc.sync.dma_start(out=xtm[:], in_=xtm_d[:])
            xtt = cpool.tile([17, R], f32, tag="xtt")
            nc.sync.dma_start(out=xtt[:], in_=xtt_d[:])

            w1 = []
            for m in range(4):
                t = cpool.tile([128, 768], f32, tag=f"w1_{m}")
                nc.sync.dma_start(out=t[:], in_=w1m_d[m])
                w1.append(t)
            w1t = cpool.tile([17, 512], f32, tag="w1t")
            nc.sync.dma_start(out=w1t[:], in_=w1t_d[:])

            w2 = []
            for m in range(2):
                t = cpool.tile([128, 512], f32, tag=f"w2_{m}")
                nc.sync.dma_start(out=t[:], in_=w2m_d[m])
                w2.append(t)
            b2x = cpool.tile([1, 257], f32, tag="b2x")
            nc.sync.dma_start(out=b2x[:], in_=b2x_d[:])
            w3x = cpool.tile([128, 2], f32, tag="w3x")
            nc.sync.dma_start(out=w3x[:], in_=w3x_d[:])

            ones = cpool.tile([1, R], f32, tag="ones")
            nc.vector.memset(ones[:], 1.0)

            # L1: d1T[m] = lrelu(W1m.T @ x + b1)   [128, R] per m-chunk
            d1 = []
            for m in range(4):
                ps = ppool.tile([128, R], f32, tag="ps1")
                for c in range(6):
                    nc.tensor.matmul(
                        ps[:], w1[m][:, 128 * c:128 * c + 128],
                        xtm[:, R * c:R * c + R],
                        start=(c == 0), stop=False)
                nc.tensor.matmul(
                    ps[:], w1t[:, 128 * m:128 * m + 128], xtt[:],
                    start=False, stop=True)
                t = wpool.tile([128, R], f32, tag="t1")
                nc.vector.tensor_scalar_mul(t[:], ps[:], 0.2)
                d = cpool.tile([128, R], f32, tag=f"d1_{m}")
                nc.vector.tensor_max(d[:], ps[:], t[:])
                d1.append(d)

            # L2: d2T[m2] = lrelu(W2m2.T @ d1 + b2)   [128, R]
            d2 = []
            for m2 in range(2):
                ps = ppool.tile([128, R], f32, tag="ps2")
                for c2 in range(4):
                    nc.tensor.matmul(
                        ps[:], w2[m2][:, 128 * c2:128 * c2 + 128], d1[c2][:],
                        start=(c2 == 0), stop=False)
                nc.tensor.matmul(
                    ps[:], b2x[0:1, 128 * m2:128 * m2 + 128], ones[:],
                    start=False, stop=True)
                t = wpool.tile([128, R], f32, tag="t2")
                nc.vector.tensor_scalar_mul(t[:], ps[:], 0.2)
                d = cpool.tile([128, R], f32, tag=f"d2_{m2}")
                nc.vector.tensor_max(d[:], ps[:], t[:])
                d2.append(d)

            # L3: out = sigmoid(w3 . d2 + b3)   [1, R]
            ps3 = ppool.tile([1, R], f32, tag="ps3")
            nc.tensor.matmul(ps3[:], w3x[:, 0:1], d2[0][:],
                             start=True, stop=False)
            nc.tensor.matmul(ps3[:], w3x[:, 1:2], d2[1][:],
                             start=False, stop=False)
            nc.tensor.matmul(ps3[:], b2x[0:1, 256:257], ones[:],
                             start=False, stop=True)
            ob = cpool.tile([1, R], f32, tag="ob")
            nc.scalar.activation(ob[:], ps3[:],
                                 mybir.ActivationFunctionType.Sigmoid)
            nc.sync.dma_start(out=out_d[:], in_=ob[:])

    nc.compile()
    return nc


def _get_nc(R: int):
    if R not in _compiled:
        _compiled[R] = _build_nc(R)
    return _compiled[R]


def _pack_weights(W1, b1, W2, b2, W3, b3):
    f = np.float32
    # w1m[m, p, c*128+j] = W1[128m+j, 128c+p]
    w1m = np.ascontiguousarray(
        W1[:, :768].reshape(4, 128, 6, 128).transpose(0, 3, 2, 1)
        .reshape(4, 128, 768), dtype=f)
    # w1t[p, o] = W1[o, 768+p] (p<16); row 16 = b1
    w1t = np.empty((17, 512), dtype=f)
    w1t[:16] = W1[:, 768:784].T
    w1t[16] = b1
    # w2m[m2, p, c2*128+j] = W2[128m2+j, 128c2+p]
    w2m = np.ascontiguousarray(
        W2.reshape(2, 128, 4, 128).transpose(0, 3, 2, 1).reshape(2, 128, 512),
        dtype=f)
    b2x = np.empty((1, 257), dtype=f)
    b2x[0, :256] = b2
    b2x[0, 256] = b3[0]
    w3x = np.ascontiguousarray(W3[0].reshape(2, 128).T, dtype=f)
    return w1m, w1t, w2m, b2x, w3x


def _pack_x(rows_c: np.ndarray, R: int):
    # rows_c: [R, 784] fp32 -> xtm [128, 6R], xtt [17, R]
    xtm = np.ascontiguousarray(
        rows_c[:, :768].reshape(R, 6, 128).transpose(2, 1, 0).reshape(128, 6 * R),
        dtype=np.float32)
    xtt = np.empty((17, R), dtype=np.float32)
    xtt[:16] = rows_c[:, 768:784].T
    xtt[16] = 1.0
    return xtm, xtt


def _run(rows: np.ndarray, R: int) -> np.ndarray:
    """rows: [N_CORES*R, 784] fp32 -> [N_CORES*R] fp32 MLP outputs."""
    from concourse.bass_utils import run_bass_kernel_spmd

    nc = _get_nc(R)
    w1m, w1t, w2m, b2x, w3x = _run._weights
    in_maps = []
    for c in range(N_CORES):
        xtm, xtt = _pack_x(rows[c * R:(c + 1) * R], R)
        in_maps.append({
            "xtm": xtm, "xtt": xtt, "w1m": w1m, "w1t": w1t,
            "w2m": w2m, "b2x": b2x, "w3x": w3x,
        })
    res = run_bass_kernel_spmd(nc, in_maps, list(range(N_CORES)))
    return np.concatenate([r["out"].reshape(R) for r in res.results])


def kernel(x, is_train_g, W1, b1, W2, b2, W3, b3):
    x = np.asarray(x, dtype=np.float32)
    _run._weights = _pack_weights(
        np.asarray(W1, np.float32), np.asarray(b1, np.float32),
        np.asarray(W2, np.float32), np.asarray(b2, np.float32),
        np.asarray(W3, np.float32), np.asarray(b3, np.float32))
    if int(is_train_g):
        rows = np.ascontiguousarray(x[:, 0, :])          # [256, 784]
        out = _run(rows, BATCH // N_CORES)               # [256]
        return out.reshape(BATCH, 1)
    else:
        rows = np.ascontiguousarray(x.reshape(BATCH * NC_LVL, D_IN))
        out = _run(rows, BATCH * NC_LVL // N_CORES)      # [1024]
        return out.reshape(BATCH, NC_LVL, 1)
